# revision 16
# baseline (speedup 1.0000x reference)
"""Trainium2 Bass kernel: disaggregated attention (prefill + decode), 8-core SPMD.

Sharding: tensor-parallel over heads. Each of the 8 cores gets 2 of the 16
heads (its slices of wq/wk/wv/wo as kernel inputs), runs an IDENTICAL program:
  - project Q^T/K^T (head-transposed layout) and V for its 2 heads
  - causal attention in S^T layout (scores-transposed), no-max softmax
    (logits are O(1) here, exp cannot overflow; masked entries get
    exp(x - 30000) == 0 exactly)
  - partial output projection against its wo slice -> [B*S, D] fp32 partial
  - decode: one new token appended to the SBUF-resident KV, same head shard
Host: transposes x once (layout prep), casts to bf16, slices weights per
core, then sums the 8 fp32 partials (the output-projection all-reduce is the
unshard step).
"""

import sys

import numpy as np

for _p in ("/opt/trn_rl_repo", "/root/.axon_site/_ro/trn_rl_repo"):
    if _p not in sys.path:
        sys.path.insert(0, _p)

import ml_dtypes  # noqa: E402

import concourse.bass as bass  # noqa: E402
import concourse.bacc as bacc  # noqa: E402
import concourse.mybir as mybir  # noqa: E402
from concourse.bass_utils import run_bass_kernel_spmd  # noqa: E402
from concourse.tile import TileContext  # noqa: E402

BF16 = ml_dtypes.bfloat16

# Problem constants (hardcoded per contract)
B, S, D, N, H = 2, 2048, 2048, 16, 128
NCORES = 8
HL = N // NCORES            # local heads per core = 2
NH_L = HL * H               # 256 local head dims
T = B * S                   # 4096 total tokens
DT = D // 128               # 16 contraction tiles
SCALE = 1.0 / float(np.sqrt(H))
MASK_PRE = -30000.0 / SCALE  # additive pre-scale mask -> exp(-30000) == 0

_COMPILED = {}


def _build_nc():
    fp32 = mybir.dt.float32
    bf16 = mybir.dt.bfloat16
    Exp = mybir.ActivationFunctionType.Exp

    nc = bacc.Bacc()
    xT_d = nc.declare_dram_parameter("xT", [D, T], bf16, isOutput=False)
    xnT_d = nc.declare_dram_parameter("xnT", [D, B], bf16, isOutput=False)
    wq_d = nc.declare_dram_parameter("wq", [D, NH_L], bf16, isOutput=False)
    wk_d = nc.declare_dram_parameter("wk", [D, NH_L], bf16, isOutput=False)
    wv_d = nc.declare_dram_parameter("wv", [D, NH_L], bf16, isOutput=False)
    woT_d = nc.declare_dram_parameter("woT", [NH_L, D], bf16, isOutput=False)
    mask_d = nc.declare_dram_parameter("maskpre", [512, 512], fp32, isOutput=False)
    opre_d = nc.declare_dram_parameter("o_pre", [T, D], fp32, isOutput=True)
    odec_d = nc.declare_dram_parameter("o_dec", [B, D], fp32, isOutput=True)

    with TileContext(nc) as tc:
        with (
            tc.tile_pool(name="pers", bufs=1) as pers,
            tc.tile_pool(name="xc", bufs=2) as xcp,
            tc.tile_pool(name="pt", bufs=3) as ptp,
            tc.tile_pool(name="fin", bufs=2) as finp,
            tc.tile_pool(name="ps_a", bufs=2, space="PSUM") as ps_a,
            tc.tile_pool(name="ps_s", bufs=2, space="PSUM") as ps_s,
            tc.tile_pool(name="ps_o", bufs=2, space="PSUM") as ps_o,
            tc.tile_pool(name="ps_l", bufs=2, space="PSUM") as ps_l,
        ):
            # ---- persistent SBUF tensors ----
            # Per-dtile tiles so each load is a single-queue DMA and no
            # consumer matmul exceeds the ISA sync-wait budget.
            wq_t = [pers.tile([128, NH_L], bf16, tag=f"wq{i}", name=f"wq{i}") for i in range(DT)]
            wk_t = [pers.tile([128, NH_L], bf16, tag=f"wk{i}", name=f"wk{i}") for i in range(DT)]
            wv_t = [pers.tile([128, NH_L], bf16, tag=f"wv{i}", name=f"wv{i}") for i in range(DT)]
            wo_t = [pers.tile([128, D], bf16, tag=f"wo{i}", name=f"wo{i}") for i in range(HL)]
            mk_t = [pers.tile([128, 512], fp32, tag=f"mk{i}", name=f"mk{i}") for i in range(4)]
            xn_t = [pers.tile([128, B], bf16, tag=f"xn{i}", name=f"xn{i}") for i in range(DT)]
            QT_sb = pers.tile([128, B * HL * S], bf16, tag="QT")
            KT_sb = pers.tile([128, B * HL * S], bf16, tag="KT")
            V_sb = pers.tile([128, (T // 128) * NH_L], bf16, tag="V")
            OnT_sb = pers.tile([128, B * HL * S], bf16, tag="OnT")
            ones_bf = pers.tile([128, 1], bf16, tag="ones_bf")
            ones_f1 = pers.tile([1, 128], fp32, tag="ones_f1")
            ones_fc = pers.tile([128, 1], fp32, tag="ones_fc")
            q1_sb = pers.tile([128, HL * B], bf16, tag="q1")
            k1_sb = pers.tile([128, HL * B], bf16, tag="k1")
            v1_sb = pers.tile([128, HL * B], bf16, tag="v1")
            o1n_sb = pers.tile([128, HL * B], bf16, tag="o1n")

            nc.vector.memset(ones_bf[:], 1.0)
            nc.vector.memset(ones_f1[:], 1.0)
            nc.vector.memset(ones_fc[:], 1.0)

            # ---- load weights / mask / x_new (one single-block DMA each) ----
            for i in range(DT):
                nc.sync.dma_start(
                    out=wq_t[i][:], in_=wq_d[i * 128 : (i + 1) * 128, :]
                )
                nc.sync.dma_start(
                    out=wk_t[i][:], in_=wk_d[i * 128 : (i + 1) * 128, :]
                )
                nc.sync.dma_start(
                    out=wv_t[i][:], in_=wv_d[i * 128 : (i + 1) * 128, :]
                )
                nc.sync.dma_start(
                    out=xn_t[i][:], in_=xnT_d[i * 128 : (i + 1) * 128, :]
                )
            for i in range(HL):
                nc.sync.dma_start(
                    out=wo_t[i][:], in_=woT_d[i * 128 : (i + 1) * 128, :]
                )
            for i in range(4):
                nc.sync.dma_start(
                    out=mk_t[i][:], in_=mask_d[i * 128 : (i + 1) * 128, :]
                )

            # ---- Phase 1: projections (stream xT in 8 chunks of 512 tokens) ----
            for ch in range(T // 512):
                b = ch // 4
                xcs = [
                    xcp.tile([128, 512], bf16, tag=f"xc{dt}", name=f"xc{dt}") for dt in range(DT)
                ]
                for dt in range(DT):
                    nc.gpsimd.dma_start(
                        out=xcs[dt][:],
                        in_=xT_d[
                            dt * 128 : (dt + 1) * 128, ch * 512 : (ch + 1) * 512
                        ],
                    )
                for hl in range(HL):
                    col = (hl * B + b) * S + (ch % 4) * 512
                    psq = ps_a.tile([128, 512], fp32, tag="pa")
                    for dt in range(DT):
                        nc.tensor.matmul(
                            psq[:],
                            wq_t[dt][:, hl * H : (hl + 1) * H],
                            xcs[dt][:],
                            start=(dt == 0),
                            stop=(dt == DT - 1),
                        )
                    nc.vector.tensor_copy(out=QT_sb[:, col : col + 512], in_=psq[:])
                    psk = ps_a.tile([128, 512], fp32, tag="pa")
                    for dt in range(DT):
                        nc.tensor.matmul(
                            psk[:],
                            wk_t[dt][:, hl * H : (hl + 1) * H],
                            xcs[dt][:],
                            start=(dt == 0),
                            stop=(dt == DT - 1),
                        )
                    nc.vector.tensor_copy(out=KT_sb[:, col : col + 512], in_=psk[:])
                for vt in range(4):
                    g = ch * 4 + vt
                    psv = ps_a.tile([128, NH_L], fp32, tag="pa")
                    for dt in range(DT):
                        nc.tensor.matmul(
                            psv[:],
                            xcs[dt][:, vt * 128 : (vt + 1) * 128],
                            wv_t[dt][:],
                            start=(dt == 0),
                            stop=(dt == DT - 1),
                        )
                    nc.vector.tensor_copy(
                        out=V_sb[:, g * NH_L : (g + 1) * NH_L], in_=psv[:]
                    )

            # ---- Phase 2: causal attention (S^T layout, no-max softmax) ----
            for b in range(B):
                for hl in range(HL):
                    base = (hl * B + b) * S
                    for qc in range(4):
                        qcol = base + qc * 512
                        nkb = 4 * qc + 4
                        o_ps = ps_o.tile([128, 512], fp32, tag="po")
                        l_ps = ps_l.tile([1, 512], fp32, tag="pl")
                        for kb in range(nkb):
                            s_ps = ps_s.tile([128, 512], fp32, tag="ps")
                            nc.tensor.matmul(
                                s_ps[:],
                                KT_sb[:, base + kb * 128 : base + (kb + 1) * 128],
                                QT_sb[:, qcol : qcol + 512],
                                start=True,
                                stop=True,
                            )
                            r = kb - 4 * qc
                            if r >= 0:
                                nc.vector.tensor_add(
                                    out=s_ps[:],
                                    in0=s_ps[:],
                                    in1=mk_t[r][:],
                                )
                            pt = ptp.tile([128, 512], bf16, tag="pt")
                            nc.scalar.activation(
                                out=pt[:], in_=s_ps[:], func=Exp, scale=SCALE
                            )
                            g = b * 16 + kb
                            nc.tensor.matmul(
                                o_ps[:],
                                V_sb[:, g * NH_L + hl * H : g * NH_L + (hl + 1) * H],
                                pt[:],
                                start=(kb == 0),
                                stop=(kb == nkb - 1),
                            )
                            nc.tensor.matmul(
                                l_ps[:],
                                ones_bf[:],
                                pt[:],
                                start=(kb == 0),
                                stop=(kb == nkb - 1),
                            )
                        linv = finp.tile([1, 512], fp32, tag="linv")
                        nc.vector.reciprocal(out=linv[:], in_=l_ps[:])
                        bc_ps = ps_l.tile([128, 512], fp32, tag="pl")
                        nc.tensor.matmul(
                            bc_ps[:], ones_f1[:], linv[:], start=True, stop=True
                        )
                        bc_sb = finp.tile([128, 512], fp32, tag="bc")
                        nc.vector.tensor_copy(out=bc_sb[:], in_=bc_ps[:])
                        nc.vector.tensor_mul(
                            out=OnT_sb[:, qcol : qcol + 512],
                            in0=o_ps[:],
                            in1=bc_sb[:],
                        )

            # ---- Phase 3: partial output projection -> DRAM fp32 ----
            for b in range(B):
                for tt in range(S // 128):
                    for dc in range(4):
                        ps = ps_a.tile([128, 512], fp32, tag="pa")
                        for hl in range(HL):
                            nc.tensor.matmul(
                                ps[:],
                                OnT_sb[
                                    :,
                                    (hl * B + b) * S + tt * 128 : (hl * B + b) * S
                                    + (tt + 1) * 128,
                                ],
                                wo_t[hl][:, dc * 512 : (dc + 1) * 512],
                                start=(hl == 0),
                                stop=(hl == HL - 1),
                            )
                        ost = finp.tile([128, 512], fp32, tag="ost")
                        nc.vector.tensor_copy(out=ost[:], in_=ps[:])
                        nc.gpsimd.dma_start(
                            out=opre_d[
                                b * S + tt * 128 : b * S + (tt + 1) * 128,
                                dc * 512 : (dc + 1) * 512,
                            ],
                            in_=ost[:],
                        )

            # ---- Phase 4: decode (new token, full cache + new entry) ----
            for hl in range(HL):
                psq = ps_a.tile([128, B], fp32, tag="pa")
                psk = ps_a.tile([128, B], fp32, tag="pa")
                for dt in range(DT):
                    nc.tensor.matmul(
                        psq[:],
                        wq_t[dt][:, hl * H : (hl + 1) * H],
                        xn_t[dt][:],
                        start=(dt == 0),
                        stop=(dt == DT - 1),
                    )
                for dt in range(DT):
                    nc.tensor.matmul(
                        psk[:],
                        wk_t[dt][:, hl * H : (hl + 1) * H],
                        xn_t[dt][:],
                        start=(dt == 0),
                        stop=(dt == DT - 1),
                    )
                psv = ps_a.tile([128, B], fp32, tag="pa")
                for dt in range(DT):
                    nc.tensor.matmul(
                        psv[:],
                        wv_t[dt][:, hl * H : (hl + 1) * H],
                        xn_t[dt][:],
                        start=(dt == 0),
                        stop=(dt == DT - 1),
                    )
                nc.vector.tensor_copy(out=q1_sb[:, hl * B : (hl + 1) * B], in_=psq[:])
                nc.vector.tensor_copy(out=k1_sb[:, hl * B : (hl + 1) * B], in_=psk[:])
                nc.vector.tensor_copy(out=v1_sb[:, hl * B : (hl + 1) * B], in_=psv[:])

            for b in range(B):
                for hl in range(HL):
                    base = (hl * B + b) * S
                    c1 = hl * B + b
                    s1 = ps_s.tile([128, 16], fp32, tag="ps")
                    for kb in range(16):
                        nc.tensor.matmul(
                            s1[:, kb : kb + 1],
                            KT_sb[:, base + kb * 128 : base + (kb + 1) * 128],
                            q1_sb[:, c1 : c1 + 1],
                            start=True,
                            stop=True,
                        )
                    sn = ps_l.tile([1, 1], fp32, tag="pl")
                    nc.tensor.matmul(
                        sn[:],
                        k1_sb[:, c1 : c1 + 1],
                        q1_sb[:, c1 : c1 + 1],
                        start=True,
                        stop=True,
                    )
                    p1 = ptp.tile([128, 16], bf16, tag="pt")
                    nc.scalar.activation(out=p1[:], in_=s1[:], func=Exp, scale=SCALE)
                    en = finp.tile([1, 1], fp32, tag="en")
                    nc.scalar.activation(out=en[:], in_=sn[:], func=Exp, scale=SCALE)
                    l1c = finp.tile([128, 1], fp32, tag="l1c")
                    nc.vector.reduce_sum(
                        out=l1c[:], in_=p1[:], axis=mybir.AxisListType.X
                    )
                    l1t_ps = ps_l.tile([1, 1], fp32, tag="pl")
                    nc.tensor.matmul(
                        l1t_ps[:], l1c[:], ones_fc[:], start=True, stop=True
                    )
                    l1t = finp.tile([1, 1], fp32, tag="l1t")
                    nc.vector.tensor_add(out=l1t[:], in0=l1t_ps[:], in1=en[:])
                    inv1 = finp.tile([1, 1], fp32, tag="inv1")
                    nc.vector.reciprocal(out=inv1[:], in_=l1t[:])
                    o1_ps = ps_o.tile([128, 1], fp32, tag="po")
                    for kb in range(16):
                        g = b * 16 + kb
                        nc.tensor.matmul(
                            o1_ps[:],
                            V_sb[:, g * NH_L + hl * H : g * NH_L + (hl + 1) * H],
                            p1[:, kb : kb + 1],
                            start=(kb == 0),
                            stop=(kb == 15),
                        )
                    bc1_ps = ps_l.tile([128, 1], fp32, tag="pl")
                    nc.tensor.matmul(
                        bc1_ps[:], ones_f1[:], inv1[:], start=True, stop=True
                    )
                    bc1 = finp.tile([128, 1], fp32, tag="bc1")
                    nc.vector.tensor_copy(out=bc1[:], in_=bc1_ps[:])
                    en_ps = ps_l.tile([128, 1], fp32, tag="pl")
                    nc.tensor.matmul(
                        en_ps[:], ones_f1[:], en[:], start=True, stop=True
                    )
                    envv = finp.tile([128, 1], fp32, tag="envv")
                    nc.vector.tensor_mul(
                        out=envv[:], in0=en_ps[:], in1=v1_sb[:, c1 : c1 + 1]
                    )
                    o1s = finp.tile([128, 1], fp32, tag="o1s")
                    nc.vector.tensor_add(out=o1s[:], in0=o1_ps[:], in1=envv[:])
                    nc.vector.tensor_mul(
                        out=o1n_sb[:, c1 : c1 + 1], in0=o1s[:], in1=bc1[:]
                    )

            for dc in range(4):
                pd = ps_a.tile([B, 512], fp32, tag="pa")
                for hl in range(HL):
                    nc.tensor.matmul(
                        pd[:],
                        o1n_sb[:, hl * B : (hl + 1) * B],
                        wo_t[hl][:, dc * 512 : (dc + 1) * 512],
                        start=(hl == 0),
                        stop=(hl == HL - 1),
                    )
                dst = finp.tile([B, 512], fp32, tag="dst")
                nc.vector.tensor_copy(out=dst[:], in_=pd[:])
                nc.gpsimd.dma_start(
                    out=odec_d[:, dc * 512 : (dc + 1) * 512], in_=dst[:]
                )

    nc.compile()
    return nc


def _prep_inputs(x, x_new, wq, wk, wv, wo):
    xT = np.ascontiguousarray(
        x.astype(np.float32).transpose(2, 0, 1).reshape(D, T)
    ).astype(BF16)
    xnT = np.ascontiguousarray(x_new.astype(np.float32).reshape(B, D).T).astype(BF16)
    # pre-scale additive causal mask for the 4 diagonal offsets
    r = np.arange(512)[:, None] % 128 + (np.arange(512)[:, None] // 128) * 128
    row = np.arange(512)[:, None]
    col = np.arange(512)[None, :]
    mask = np.where(row % 128 + (row // 128) * 128 <= col, 0.0, MASK_PRE)
    # rows are (r, t) flattened: global row = r*128 + t; visible iff r*128+t <= j
    mask = np.where(row <= col, 0.0, MASK_PRE).astype(np.float32)

    in_maps = []
    for c in range(NCORES):
        h0 = c * HL
        wq_c = np.ascontiguousarray(
            wq[:, h0 : h0 + HL, :].reshape(D, NH_L)
        ).astype(BF16)
        wk_c = np.ascontiguousarray(
            wk[:, h0 : h0 + HL, :].reshape(D, NH_L)
        ).astype(BF16)
        wv_c = np.ascontiguousarray(
            wv[:, h0 : h0 + HL, :].reshape(D, NH_L)
        ).astype(BF16)
        woT_c = np.ascontiguousarray(
            wo[:, h0 : h0 + HL, :].transpose(1, 2, 0).reshape(NH_L, D)
        ).astype(BF16)
        in_maps.append(
            {
                "xT": xT,
                "xnT": xnT,
                "wq": wq_c,
                "wk": wk_c,
                "wv": wv_c,
                "woT": woT_c,
                "maskpre": mask,
            }
        )
    return in_maps


def kernel(x, x_new, wq, wk, wv, wo, _trace=False):
    if "nc" not in _COMPILED:
        _COMPILED["nc"] = _build_nc()
    nc = _COMPILED["nc"]
    in_maps = _prep_inputs(x, x_new, wq, wk, wv, wo)
    res = run_bass_kernel_spmd(
        nc, in_maps, list(range(NCORES)), trace=_trace
    )
    pre = np.zeros((T, D), np.float32)
    dec = np.zeros((B, D), np.float32)
    for c in range(NCORES):
        pre += res.results[c]["o_pre"]
        dec += res.results[c]["o_dec"]
    _COMPILED["last_exec_time_ns"] = res.exec_time_ns
    _COMPILED["last_profile"] = res.profile_json
    return pre.reshape(B, S, D), dec.reshape(B, 1, D)
